# revision 1
# baseline (speedup 1.0000x reference)
"""Trainium2 Bass kernel for nn_ContrastiveLoss (SimCLR + spatial contrastive loss).

Strategy (8-core data parallel):
  - Host: L2-normalize z1/z2/embeddings (fp32), build transposed bf16 operand
    tables, gather anchor rows, compute fp32 positive-pair dots.
  - Device (per core): bf16 matmuls of its 1024 simclr rows and 512 spatial
    rows against the full 8192-column tables; fused exp(x/T) + row-sum on the
    ACT engine straight out of PSUM; a tiny PE Gram matmul per row-tile whose
    diagonal reproduces bit-exactly the self-similarity terms, which are
    exp'd identically and returned as per-row corrections.
  - Host: sum_exp = S_raw - corr (exact diagonal/anchor-column removal),
    log, subtract positives, mean-reduce -> [2] losses.

Self-contained: hardcodes shapes from the problem spec.
"""
import sys

for _p in ("/opt/trn_rl_repo", "/root/.axon_site/_ro/trn_rl_repo"):
    if _p not in sys.path:
        sys.path.insert(0, _p)

import numpy as np
import ml_dtypes

import concourse.tile as tile
from concourse import bacc, mybir
from concourse.bass_utils import run_bass_kernel_spmd

TEMPERATURE = 0.07
B = 4096     # simclr batch
D = 256      # projection dim
N = 8192     # num cells (spatial table rows, also 2B simclr table rows)
P = 4096     # num spatial pairs
NCORES = 8
SR = B // NCORES          # 512 simclr pair-rows per core (=> 1024 sim rows)
PR = P // NCORES          # 512 spatial rows per core
RT_SIMCLR = (2 * SR) // 128   # 8 row-tiles
RT_SPATIAL = PR // 128        # 4 row-tiles
RT_TOTAL = RT_SIMCLR + RT_SPATIAL  # 12
NCHUNK = N // 512         # 16 column chunks of 512
NGROUP = 4                # psum groups of 2048 columns
F32 = mybir.dt.float32
BF16 = mybir.dt.bfloat16

_CACHE = {}


def _build_nc():
    nc = bacc.Bacc("TRN2", target_bir_lowering=False)

    zT = nc.dram_tensor("zT", [128, 2, N], BF16, kind="ExternalInput")
    eT = nc.dram_tensor("eT", [128, 2, N], BF16, kind="ExternalInput")
    zTl = nc.dram_tensor("zTl", [128, 2, 2 * SR], BF16, kind="ExternalInput")
    aTl = nc.dram_tensor("aTl", [128, 2, PR], BF16, kind="ExternalInput")
    ident = nc.dram_tensor("ident", [128, 128], F32, kind="ExternalInput")

    sraw_o = nc.dram_tensor("sraw", [128, RT_TOTAL], F32, kind="ExternalOutput")
    corr_o = nc.dram_tensor("corr", [128, RT_TOTAL], F32, kind="ExternalOutput")

    inv_t = float(1.0 / np.float32(TEMPERATURE))

    with tile.TileContext(nc) as tc:
        with (
            tc.tile_pool(name="tabs", bufs=1) as tabs,
            tc.tile_pool(name="psum", bufs=2, space="PSUM") as psum,
            tc.tile_pool(name="scr", bufs=2) as scrp,
            tc.tile_pool(name="small", bufs=1) as small,
            tc.tile_pool(name="tmp", bufs=4) as tmpp,
        ):
            # Small operand tables first so PE can start (grams) immediately;
            # big tables split per 2048-column group so the first main matmul
            # group only waits on its own 1MB chunk.
            zTl_t = tabs.tile([128, 2, 2 * SR], BF16)
            aTl_t = tabs.tile([128, 2, PR], BF16)
            ident_t = small.tile([128, 128], F32)
            # Group 0 of zT lives in four 512-column sub-tiles so the very
            # first matmuls/exps only wait for 0.25MB of DMA, not 3.2MB.
            zT_c = [tabs.tile([128, 2, 512], BF16, name=f"zTc{j}")
                    for j in range(4)]
            zT_g = [None] + [tabs.tile([128, 2, 2048], BF16, name=f"zTg{g}")
                             for g in range(1, NGROUP)]
            eT_g = [tabs.tile([128, 2, 2048], BF16, name=f"eTg{g}")
                    for g in range(NGROUP)]
            # Load order = consumption order: lhsT slices, then the first rhs
            # chunks (critical path of the first matmul group), then the rest.
            nc.sync.dma_start(zTl_t[:], zTl[:])
            nc.sync.dma_start(aTl_t[:], aTl[:])
            for j in range(4):
                nc.sync.dma_start(zT_c[j][:], zT[:, :, j * 512:(j + 1) * 512])
            nc.sync.dma_start(ident_t[:], ident[:])
            for g in range(1, NGROUP):
                nc.sync.dma_start(zT_g[g][:], zT[:, :, g * 2048:(g + 1) * 2048])
            for g in range(NGROUP):
                nc.sync.dma_start(eT_g[g][:], eT[:, :, g * 2048:(g + 1) * 2048])

            sraw_t = small.tile([128, RT_TOTAL], F32)
            corr_t = small.tile([128, RT_TOTAL], F32)

            def lhsT_pair(rt):
                lh, li = (zTl_t, rt) if rt < RT_SIMCLR else (aTl_t, rt - RT_SIMCLR)
                return (lh[:, 0, li * 128:(li + 1) * 128],
                        lh[:, 1, li * 128:(li + 1) * 128])

            # All Gram diagonals up front: the diagonal of lhsT.T@lhsT is
            # bitwise-identical to the main matmul's self-similarity element
            # for each row; exp'd identically it cancels those terms exactly.
            pgr = psum.tile([128, 2048], F32, tag="big")
            for grt in range(RT_TOTAL):
                l0, l1 = lhsT_pair(grt)
                nc.tensor.matmul(pgr[:, grt * 128:(grt + 1) * 128],
                                 l0, l0, start=True, stop=False)
                nc.tensor.matmul(pgr[:, grt * 128:(grt + 1) * 128],
                                 l1, l1, start=False, stop=True)
            gd_all = tmpp.tile([128, RT_TOTAL, 128], F32, tag="gd")
            for grt in range(RT_TOTAL):
                nc.vector.tensor_tensor(
                    gd_all[:, grt, :],
                    pgr[:, grt * 128:(grt + 1) * 128],
                    ident_t[:], mybir.AluOpType.mult,
                )

            gdv_all = tmpp.tile([128, RT_TOTAL], F32, tag="gdv")
            nc.vector.tensor_reduce(
                gdv_all[:], gd_all[:],
                axis=mybir.AxisListType.X, op=mybir.AluOpType.add,
            )
            nc.scalar.activation(
                corr_t[:], gdv_all[:],
                mybir.ActivationFunctionType.Exp, scale=inv_t,
            )
            nc.sync.dma_start(corr_o[:], corr_t[:])

            # Persistent per-(row-tile, group) partial sums; zeroed once so
            # the final reduce can span unused slots of the fine-grained rt0.
            part_all = small.tile([128, RT_TOTAL, NGROUP + 3], F32)
            nc.vector.memset(part_all[:], 0.0)

            def emit_unit(rt, g):
                """8 matmuls + exp(accum) for one (row-tile, 2048-col group)."""
                lhsT0, lhsT1 = lhsT_pair(rt)
                simclr = rt < RT_SIMCLR
                fine = rt == 0 and g == 0
                pg = psum.tile([128, 2048], F32, tag="big")
                mm_order = ([(kc, cc) for cc in range(4) for kc in range(2)]
                            if fine else
                            [(kc, cc) for kc in range(2) for cc in range(4)])
                for kc, cc in mm_order:
                    lz = lhsT0 if kc == 0 else lhsT1
                    if simclr and g == 0:
                        rhs = zT_c[cc][:, kc, :]
                    else:
                        tab = zT_g[g] if simclr else eT_g[g]
                        rhs = tab[:, kc, cc * 512:(cc + 1) * 512]
                    nc.tensor.matmul(
                        pg[:, cc * 512:(cc + 1) * 512], lz, rhs,
                        start=(kc == 0), stop=(kc == 1),
                    )
                # exp output is dead (only accum_out matters): write it
                # in-place over the PSUM bank.
                if fine:
                    for cc in range(4):
                        nc.scalar.activation(
                            pg[:, cc * 512:(cc + 1) * 512],
                            pg[:, cc * 512:(cc + 1) * 512],
                            mybir.ActivationFunctionType.Exp,
                            scale=inv_t, accum_out=part_all[:, rt, cc:cc + 1],
                        )
                else:
                    # rt0 g1-3 shift past the four fine-grained g0 slots
                    ps = g + 3 if rt == 0 else g
                    nc.scalar.activation(
                        pg[:], pg[:], mybir.ActivationFunctionType.Exp,
                        scale=inv_t, accum_out=part_all[:, rt, ps:ps + 1],
                    )

            # Simclr sweeps group-major: once the first 1MB column group has
            # arrived, all 8 row-tiles can run against it, so ACT never
            # starves during the remaining table DMA. Spatial runs after
            # (eT is fully resident long before it starts).
            for g in range(NGROUP):
                for rt in range(RT_SIMCLR):
                    emit_unit(rt, g)
            for rt in range(RT_SIMCLR, RT_TOTAL):
                for g in range(NGROUP):
                    emit_unit(rt, g)

            nc.vector.tensor_reduce(
                sraw_t[:], part_all[:],
                axis=mybir.AxisListType.X, op=mybir.AluOpType.add,
            )

            nc.sync.dma_start(sraw_o[:], sraw_t[:])

    nc.finalize()
    return nc


def _l2norm(x):
    n = np.maximum(np.linalg.norm(x.astype(np.float32), axis=1, keepdims=True), 1e-12)
    return (x.astype(np.float32) / n).astype(np.float32)


def _pack_T(x):
    """[R, D=256] fp32 -> transposed bf16 operand table [128, 2, R]."""
    xT = np.ascontiguousarray(x.T)                      # [256, R]
    return np.ascontiguousarray(
        xT.reshape(2, 128, xT.shape[1]).transpose(1, 0, 2)
    ).astype(ml_dtypes.bfloat16)


def prepare(z1, z2, embeddings, anchor_idx, neighbor_idx):
    """Host-side prep: returns (in_maps, host_ctx)."""
    z1n = _l2norm(np.asarray(z1))
    z2n = _l2norm(np.asarray(z2))
    en = _l2norm(np.asarray(embeddings))
    ai = np.asarray(anchor_idx).astype(np.int64)
    ni = np.asarray(neighbor_idx).astype(np.int64)

    zcat = np.concatenate([z1n, z2n], axis=0)           # [2B, D]
    zT_p = _pack_T(zcat)                                # [128, 2, 8192] bf16
    eT_p = _pack_T(en)                                  # [128, 2, 8192] bf16
    a_rows = en[ai]                                     # [P, D] fp32
    aT_p = _pack_T(a_rows)                              # [128, 2, 4096] bf16

    # fp32 positive-pair logits (match reference semantics)
    psim = (np.sum(z1n.astype(np.float64) * z2n.astype(np.float64), axis=1)
            / np.float64(np.float32(TEMPERATURE)))      # [B]
    pos = (np.sum(a_rows.astype(np.float64) * en[ni].astype(np.float64), axis=1)
           / np.float64(np.float32(TEMPERATURE)))       # [P]
    eq = (ai == ni).astype(np.float64)                  # [P]

    ident = np.eye(128, dtype=np.float32)
    in_maps = []
    for c in range(NCORES):
        zTl_p = np.ascontiguousarray(np.concatenate(
            [zT_p[:, :, c * SR:(c + 1) * SR],
             zT_p[:, :, B + c * SR:B + (c + 1) * SR]], axis=2))  # [128,2,1024]
        aTl_p = np.ascontiguousarray(aT_p[:, :, c * PR:(c + 1) * PR])  # [128,2,512]
        in_maps.append({
            "zT": zT_p, "eT": eT_p, "zTl": zTl_p, "aTl": aTl_p, "ident": ident,
        })
    return in_maps, (psim, pos, eq)


def finish(results, host_ctx):
    """Host-side epilogue: assemble the two losses from per-core S_raw/corr."""
    psim, pos, eq = host_ctx
    terms1 = np.empty(2 * B, dtype=np.float64)
    terms2 = np.empty(P, dtype=np.float64)
    for c in range(NCORES):
        S = results[c]["sraw"].astype(np.float64).T.reshape(-1)   # [12*128], idx rt*128+p
        C = results[c]["corr"].astype(np.float64).T.reshape(-1)

        s_sim = S[:2 * SR * 1]  # first 8 tiles = 1024 rows
        c_sim = C[:2 * SR]
        sum_exp = s_sim[:2 * SR] - c_sim[:2 * SR]
        p_loc = psim[c * SR:(c + 1) * SR]
        # local rows [0,512) -> z1 part, [512,1024) -> z2 part; same positives
        terms1[c * SR:(c + 1) * SR] = np.log(sum_exp[:SR]) - p_loc
        terms1[B + c * SR:B + (c + 1) * SR] = np.log(sum_exp[SR:2 * SR]) - p_loc

        s_sp = S[2 * SR:2 * SR + PR]
        c_sp = C[2 * SR:2 * SR + PR]
        g = slice(c * PR, (c + 1) * PR)
        total = s_sp - c_sp + eq[g] * np.exp(pos[g])
        terms2[g] = np.log(total) - pos[g]

    l1 = terms1.mean()
    l2 = terms2.mean()
    return np.array([l1, l2], dtype=np.float32)


def get_nc():
    if "nc" not in _CACHE:
        _CACHE["nc"] = _build_nc()
    return _CACHE["nc"]


def kernel(z1, z2, embeddings, anchor_idx, neighbor_idx):
    in_maps, host_ctx = prepare(z1, z2, embeddings, anchor_idx, neighbor_idx)
    nc = get_nc()
    res = run_bass_kernel_spmd(nc, in_maps, list(range(NCORES)))
    return finish(res.results, host_ctx)



# revision 5
# speedup vs baseline: 1.1311x; 1.1311x over previous
"""Trainium2 Bass kernel for nn_ContrastiveLoss (SimCLR + spatial contrastive loss).

Strategy (8-core data parallel):
  - Host: L2-normalize z1/z2/embeddings, quantize 8x-scaled rows to fp8e4,
    build transposed [128, 2, N] operand tables, gather anchor rows, compute
    fp64 positive-pair dots from the unquantized values.
  - Device (per core): fp8 DoubleRow matmuls (one instruction per K=256
    contraction, 0.5 cycles/row) of its 1024 simclr rows and 512 spatial rows
    against the full 8192-column tables. The exp(x/(64T)) + row-sum work is
    split across TWO engines: the ACT engine (exact exp, accum_out) handles
    ~5/8 of the 2048-column groups, and the DVE handles the rest via a
    two-instruction custom-op chain: a cubic Taylor of e^(sigma*x) followed by
    8 chained squarings ((e^(L/256))^256, rel err ~1e-5) with a fused
    row-sum accumulator. A tiny fp8 Gram matmul per row-tile reproduces
    bit-exactly the self-similarity terms; they are exp'd by BOTH engines'
    pipelines and returned as two per-row correction variants.
  - Host: picks the correction matching the engine that exp'd each row's
    self column (exact cancellation), assembles sum_exp, log, subtracts
    positives, mean-reduces -> [2] losses.

Self-contained: hardcodes shapes from the problem spec.
"""
import sys
from operator import add as _op_add

for _p in ("/opt/trn_rl_repo", "/root/.axon_site/_ro/trn_rl_repo"):
    if _p not in sys.path:
        sys.path.insert(0, _p)

import numpy as np
import ml_dtypes

import concourse.tile as tile
from concourse import bacc, mybir
from concourse.bass_utils import run_bass_kernel_spmd

TEMPERATURE = 0.07
B = 4096     # simclr batch
D = 256      # projection dim
N = 8192     # num cells (spatial table rows, also 2B simclr table rows)
P = 4096     # num spatial pairs
NCORES = 8
SR = B // NCORES          # 512 simclr pair-rows per core (=> 1024 sim rows)
PR = P // NCORES          # 512 spatial rows per core
RT_SIMCLR = (2 * SR) // 128   # 8 row-tiles
RT_SPATIAL = PR // 128        # 4 row-tiles
RT_TOTAL = RT_SIMCLR + RT_SPATIAL  # 12
NGROUP = 4                # psum groups of 2048 columns
F32 = mybir.dt.float32
FP8 = mybir.dt.float8e4
NP_FP8 = ml_dtypes.float8_e4m3

# exp scaling: psum x = 64*(a.b) (8x-scaled fp8 rows); logit L = x*INV64T.
INV64T = float(1.0 / (64.0 * np.float32(TEMPERATURE)))
SIGMA = INV64T / 256.0    # DVE op1 computes e^(SIGMA*x), op2 raises to 256
D1 = float(SIGMA)
D2 = float(SIGMA * SIGMA / 2.0)
D3 = float(SIGMA * SIGMA * SIGMA / 6.0)

# --- engine assignment: unit u = emission order over 48 (rt, g) units ------ #
# Emission: simclr group-major (g outer, rt 0..7 inner), then spatial
# (rt 8..11 outer, g inner). DVE takes 3 of every 8 units.
UNITS_ORDERED = [(rt, g) for g in range(NGROUP) for rt in range(RT_SIMCLR)]
UNITS_ORDERED += [(rt, g) for rt in range(RT_SIMCLR, RT_TOTAL) for g in range(NGROUP)]
_DVE_PAT = {2, 5, 7}
UNIT_ENGINE = {}          # (rt, g) -> "act" | "dve"
for _i, _u in enumerate(UNITS_ORDERED):
    UNIT_ENGINE[_u] = "dve" if (_i % 8) in _DVE_PAT else "act"

_CACHE = {}


# --- custom DVE exp ops ----------------------------------------------------- #
def _register_dve_exp_ops():
    """Register two custom DVE ops (idempotent):
    EXP_P3_ANT:  out = p3(Src0)^2 with p3 = 1 + t + t^2/2 + t^3/6, t = SIGMA*x
                 folded as a Horner cubic in Src0 (C0=d3, C1=d2, C2=d1, +One).
    EXP_SQ8_ANT: out = Src0^128 via 7 chained squarings; accum_out = row sum.
    Chained: (p3(x)^2)^128 = p3(x)^256 ~ e^(256*SIGMA*x) = e^(x/(64T)).
    """
    import concourse.dve_ops as dve_ops
    from concourse.dve_ops import DveOp, OPS, CUSTOM_DVE_SPECS, _SUB_OPCODE_FOR_NAME
    from concourse.dve_spec import Spec, Src0, C0, C1, C2, One, lower
    from concourse.dve_uop import DveOpSpec

    if "EXP_P3_ANT" in CUSTOM_DVE_SPECS:
        return

    def _ref_exp_p3(in0, in1, c0, c1, c2):
        x = in0.astype(np.float32)
        m1 = (x * np.float32(c0)).astype(np.float32)
        a1 = (m1 + np.float32(c1)).astype(np.float32)
        m2 = (a1 * x).astype(np.float32)
        a2 = (m2 + np.float32(c2)).astype(np.float32)
        m3 = (a2 * x).astype(np.float32)
        p = (m3 + np.float32(1.0)).astype(np.float32)
        return (p * p).astype(np.float32)

    def _ref_exp_sq8(in0, in1, c0, c1, c2):
        y = in0.astype(np.float32)
        for _ in range(7):
            y = (y * y).astype(np.float32)
        return y, y.reshape(y.shape[0], -1).sum(axis=-1, dtype=np.float64).astype(
            np.float32
        ).reshape(y.shape[0], 1)

    m1 = Src0 * C0
    a1 = m1 + C1
    m2 = a1 * Src0
    a2 = m2 + C2
    m3 = a2 * Src0
    p = m3 + One
    spec_p3 = Spec(body=p * p, reference=_ref_exp_p3)

    y = Src0
    for _ in range(7):
        y = y * y
    spec_sq8 = Spec(body=y, accum=_op_add, reference=_ref_exp_sq8)

    ops = []
    for name, spec, perf in (
        ("EXP_P3_ANT", spec_p3, False),
        ("EXP_SQ8_ANT", spec_sq8, True),
    ):
        shas = {}
        for ver in ("v3", "v4"):
            uops = lower(spec, ver=ver)
            shas[ver] = DveOpSpec(name=name, opcode=0, uops=uops, rd1_en=False).sha(ver)
        op = DveOp(
            name,
            spec,
            subdim=False,
            uops_sha=shas,
            perf_en={"v3": perf, "v4": perf} if perf else {},
        )
        ops.append(op)

    base = max(_SUB_OPCODE_FOR_NAME.values()) + 1
    for i, op in enumerate(ops):
        OPS.append(op)
        CUSTOM_DVE_SPECS[op.name] = op.spec
        _SUB_OPCODE_FOR_NAME[op.name] = base + i
    dve_ops.EXP_P3_ANT = ops[0]
    dve_ops.EXP_SQ8_ANT = ops[1]


def _get_exp_ops():
    _register_dve_exp_ops()
    import concourse.dve_ops as dve_ops

    return dve_ops.EXP_P3_ANT, dve_ops.EXP_SQ8_ANT


def _build_nc():
    EXP_P3, EXP_SQ8 = _get_exp_ops()
    nc = bacc.Bacc("TRN2", target_bir_lowering=False)

    zT = nc.dram_tensor("zT", [128, 2, N], FP8, kind="ExternalInput")
    eT = nc.dram_tensor("eT", [128, 2, N], FP8, kind="ExternalInput")
    zTl = nc.dram_tensor("zTl", [128, 2, 2 * SR], FP8, kind="ExternalInput")
    aTl = nc.dram_tensor("aTl", [128, 2, PR], FP8, kind="ExternalInput")
    ident = nc.dram_tensor("ident", [128, 128], F32, kind="ExternalInput")

    sraw_o = nc.dram_tensor("sraw", [128, RT_TOTAL], F32, kind="ExternalOutput")
    corra_o = nc.dram_tensor("corra", [128, RT_TOTAL], F32, kind="ExternalOutput")
    corrd_o = nc.dram_tensor("corrd", [128, RT_TOTAL], F32, kind="ExternalOutput")

    with tile.TileContext(nc) as tc:
        with (
            tc.tile_pool(name="tabs", bufs=1) as tabs,
            tc.tile_pool(name="psum", bufs=2, space="PSUM") as psum,
            tc.tile_pool(name="qscr", bufs=2) as qscr,
            tc.tile_pool(name="small", bufs=1) as small,
            tc.tile_pool(name="tmp", bufs=4) as tmpp,
        ):
            # Small operand tables first so PE can start (grams) immediately;
            # big tables split per 2048-column group so the first main matmul
            # group only waits on its own chunk.
            zTl_t = tabs.tile([128, 2, 2 * SR], FP8)
            aTl_t = tabs.tile([128, 2, PR], FP8)
            ident_t = small.tile([128, 128], F32)
            # Group 0 of zT in four 512-column sub-tiles so the very first
            # matmuls/exps only wait for a small DMA.
            zT_c = [tabs.tile([128, 2, 512], FP8, name=f"zTc{j}") for j in range(4)]
            zT_g = [None] + [tabs.tile([128, 2, 2048], FP8, name=f"zTg{g}")
                             for g in range(1, NGROUP)]
            eT_g = [tabs.tile([128, 2, 2048], FP8, name=f"eTg{g}")
                    for g in range(NGROUP)]
            nc.sync.dma_start(zTl_t[:], zTl[:])
            nc.sync.dma_start(aTl_t[:], aTl[:])
            for j in range(4):
                nc.sync.dma_start(zT_c[j][:], zT[:, :, j * 512:(j + 1) * 512])
            nc.sync.dma_start(ident_t[:], ident[:])
            for g in range(1, NGROUP):
                nc.sync.dma_start(zT_g[g][:], zT[:, :, g * 2048:(g + 1) * 2048])
            for g in range(NGROUP):
                nc.sync.dma_start(eT_g[g][:], eT[:, :, g * 2048:(g + 1) * 2048])

            sraw_t = small.tile([128, RT_TOTAL], F32)
            corra_t = small.tile([128, RT_TOTAL], F32)
            corrd_t = small.tile([128, RT_TOTAL], F32)

            def lhsT_pair(rt):
                lh, li = (zTl_t, rt) if rt < RT_SIMCLR else (aTl_t, rt - RT_SIMCLR)
                return lh[:, :, li * 128:(li + 1) * 128]

            # Gram diagonals up front: diag of lhsT.T@lhsT is bitwise-identical
            # to the main matmul's self-similarity element for each row; exp'd
            # through both engine pipelines it cancels those terms exactly.
            pgr = psum.tile([128, 2048], F32, tag="big")
            for grt in range(RT_TOTAL):
                lz = lhsT_pair(grt)
                nc.tensor.matmul(pgr[:, grt * 128:(grt + 1) * 128], lz, lz,
                                 start=True, stop=True,
                                 perf_mode=mybir.MatmulPerfMode.DoubleRow)
            gd_all = tmpp.tile([128, RT_TOTAL, 128], F32, tag="gd")
            for grt in range(RT_TOTAL):
                nc.vector.tensor_tensor(
                    gd_all[:, grt, :],
                    pgr[:, grt * 128:(grt + 1) * 128],
                    ident_t[:], mybir.AluOpType.mult,
                )
            gdv_all = tmpp.tile([128, RT_TOTAL], F32, tag="gdv")
            nc.vector.tensor_reduce(
                gdv_all[:], gd_all[:],
                axis=mybir.AxisListType.X, op=mybir.AluOpType.add,
            )
            # ACT-exact correction
            nc.scalar.activation(
                corra_t[:], gdv_all[:],
                mybir.ActivationFunctionType.Exp, scale=INV64T,
            )
            nc.sync.dma_start(corra_o[:], corra_t[:])
            # DVE-pipeline correction (same two ops as the main DVE exp path)
            qcorr = tmpp.tile([128, RT_TOTAL], F32, tag="qcorr")
            nc.vector._custom_dve(EXP_P3, out=qcorr[:], in0=gdv_all[:],
                                  s0=D3, s1=D2, imm2=D1)
            nc.vector._custom_dve(EXP_SQ8, out=corrd_t[:], in0=qcorr[:])
            nc.sync.dma_start(corrd_o[:], corrd_t[:])

            # Persistent per-(row-tile, group) partial sums; zeroed once so
            # the final reduce can span unused slots of the fine-grained rt0.
            part_all = small.tile([128, RT_TOTAL, NGROUP + 3], F32)
            nc.vector.memset(part_all[:], 0.0)

            def emit_unit(rt, g):
                """4 DoubleRow matmuls + exp(accum) for one (rt, 2048-col group)."""
                lz = lhsT_pair(rt)
                simclr = rt < RT_SIMCLR
                fine = rt == 0 and g == 0
                engine = UNIT_ENGINE[(rt, g)]
                pg = psum.tile([128, 2048], F32, tag="big")
                for cc in range(4):
                    if simclr and g == 0:
                        rhs = zT_c[cc][:]
                    else:
                        tab = zT_g[g] if simclr else eT_g[g]
                        rhs = tab[:, :, cc * 512:(cc + 1) * 512]
                    nc.tensor.matmul(
                        pg[:, cc * 512:(cc + 1) * 512], lz, rhs,
                        start=True, stop=True,
                        perf_mode=mybir.MatmulPerfMode.DoubleRow,
                    )
                if engine == "act":
                    # exp output is dead (only accum_out matters): write it
                    # in-place over the PSUM bank.
                    if fine:
                        for cc in range(4):
                            nc.scalar.activation(
                                pg[:, cc * 512:(cc + 1) * 512],
                                pg[:, cc * 512:(cc + 1) * 512],
                                mybir.ActivationFunctionType.Exp,
                                scale=INV64T,
                                accum_out=part_all[:, rt, cc:cc + 1],
                            )
                    else:
                        ps = g + 3 if rt == 0 else g
                        nc.scalar.activation(
                            pg[:], pg[:], mybir.ActivationFunctionType.Exp,
                            scale=INV64T, accum_out=part_all[:, rt, ps:ps + 1],
                        )
                else:
                    ps = g + 3 if rt == 0 else g
                    q = qscr.tile([128, 2048], F32, tag="q")
                    nc.vector._custom_dve(EXP_P3, out=q[:], in0=pg[:],
                                          s0=D3, s1=D2, imm2=D1)
                    nc.vector._custom_dve(EXP_SQ8, out=q[:], in0=q[:],
                                          accum_out=part_all[:, rt, ps:ps + 1])

            # Simclr sweeps group-major: once the first column group has
            # arrived, all 8 row-tiles can run against it. Spatial runs after.
            for rt, g in UNITS_ORDERED:
                emit_unit(rt, g)

            nc.vector.tensor_reduce(
                sraw_t[:], part_all[:],
                axis=mybir.AxisListType.X, op=mybir.AluOpType.add,
            )
            nc.sync.dma_start(sraw_o[:], sraw_t[:])

    nc.finalize()
    return nc


def _l2norm(x):
    n = np.maximum(np.linalg.norm(x.astype(np.float32), axis=1, keepdims=True), 1e-12)
    return (x.astype(np.float32) / n).astype(np.float32)


def _pack_T(x8):
    """[R, D=256] fp8 -> transposed operand table [128, 2, R] (k-tile dim 1)."""
    xT = np.ascontiguousarray(x8.T)                     # [256, R]
    return np.ascontiguousarray(
        xT.reshape(2, 128, xT.shape[1]).transpose(1, 0, 2)
    )


def prepare(z1, z2, embeddings, anchor_idx, neighbor_idx):
    """Host-side prep: returns (in_maps, host_ctx)."""
    z1n = _l2norm(np.asarray(z1))
    z2n = _l2norm(np.asarray(z2))
    en = _l2norm(np.asarray(embeddings))
    ai = np.asarray(anchor_idx).astype(np.int64)
    ni = np.asarray(neighbor_idx).astype(np.int64)

    zcat8 = (np.concatenate([z1n, z2n], axis=0) * np.float32(8.0)).astype(NP_FP8)
    en8 = (en * np.float32(8.0)).astype(NP_FP8)
    zT_p = _pack_T(zcat8)                               # [128, 2, 8192] fp8
    eT_p = _pack_T(en8)                                 # [128, 2, 8192] fp8
    a8 = en8[ai]                                        # [P, D] fp8 (table rows)
    aT_p = _pack_T(a8)                                  # [128, 2, 4096] fp8

    # fp64 positive-pair logits from the unquantized values
    psim = (np.sum(z1n.astype(np.float64) * z2n.astype(np.float64), axis=1)
            / np.float64(np.float32(TEMPERATURE)))      # [B]
    pos = (np.sum(en[ai].astype(np.float64) * en[ni].astype(np.float64), axis=1)
           / np.float64(np.float32(TEMPERATURE)))       # [P]
    eq = (ai == ni).astype(np.float64)                  # [P]

    ident = np.eye(128, dtype=np.float32)
    in_maps = []
    for c in range(NCORES):
        zTl_p = np.ascontiguousarray(np.concatenate(
            [zT_p[:, :, c * SR:(c + 1) * SR],
             zT_p[:, :, B + c * SR:B + (c + 1) * SR]], axis=2))  # [128,2,1024]
        aTl_p = np.ascontiguousarray(aT_p[:, :, c * PR:(c + 1) * PR])  # [128,2,512]
        in_maps.append({
            "zT": zT_p, "eT": eT_p, "zTl": zTl_p, "aTl": aTl_p, "ident": ident,
        })
    return in_maps, (psim, pos, eq, ai)


def finish(results, host_ctx):
    """Host-side epilogue: assemble the two losses from per-core outputs."""
    psim, pos, eq, ai = host_ctx
    terms1 = np.empty(2 * B, dtype=np.float64)
    terms2 = np.empty(P, dtype=np.float64)
    for c in range(NCORES):
        S = results[c]["sraw"].astype(np.float64).T.reshape(-1)   # idx rt*128+p
        Ca = results[c]["corra"].astype(np.float64).T.reshape(-1)
        Cd = results[c]["corrd"].astype(np.float64).T.reshape(-1)

        # simclr local rows l in [0, 1024): global z-table col
        l = np.arange(2 * SR)
        gcol = np.where(l < SR, c * SR + l, B + c * SR + (l - SR))
        rt_l = l // 128
        g_of_col = gcol // 2048
        use_dve = np.array(
            [UNIT_ENGINE[(int(rt_l[i]), int(g_of_col[i]))] == "dve"
             for i in range(2 * SR)])
        corr_sim = np.where(use_dve, Cd[:2 * SR], Ca[:2 * SR])
        sum_exp = S[:2 * SR] - corr_sim
        p_loc = psim[c * SR:(c + 1) * SR]
        terms1[c * SR:(c + 1) * SR] = np.log(sum_exp[:SR]) - p_loc
        terms1[B + c * SR:B + (c + 1) * SR] = np.log(sum_exp[SR:2 * SR]) - p_loc

        # spatial local rows r in [0, 512): anchor col is data-dependent
        gsl = slice(c * PR, (c + 1) * PR)
        ai_loc = ai[gsl]
        r = np.arange(PR)
        rt_sp = RT_SIMCLR + r // 128
        g_anchor = ai_loc // 2048
        use_dve_sp = np.array(
            [UNIT_ENGINE[(int(rt_sp[i]), int(g_anchor[i]))] == "dve"
             for i in range(PR)])
        s_sp = S[2 * SR:2 * SR + PR]
        c_sp = np.where(use_dve_sp, Cd[2 * SR:2 * SR + PR], Ca[2 * SR:2 * SR + PR])
        total = s_sp - c_sp + eq[gsl] * np.exp(pos[gsl])
        terms2[gsl] = np.log(total) - pos[gsl]

    l1 = terms1.mean()
    l2 = terms2.mean()
    return np.array([l1, l2], dtype=np.float32)


def get_nc():
    if "nc" not in _CACHE:
        _CACHE["nc"] = _build_nc()
    return _CACHE["nc"]


def kernel(z1, z2, embeddings, anchor_idx, neighbor_idx):
    in_maps, host_ctx = prepare(z1, z2, embeddings, anchor_idx, neighbor_idx)
    nc = get_nc()
    res = run_bass_kernel_spmd(nc, in_maps, list(range(NCORES)))
    return finish(res.results, host_ctx)


# revision 6
# speedup vs baseline: 1.3052x; 1.1540x over previous
"""Trainium2 Bass kernel for nn_ContrastiveLoss (SimCLR + spatial contrastive loss).

Strategy (8-core data parallel):
  - Host: L2-normalize z1/z2/embeddings, quantize 8x-scaled rows to fp8e4,
    build transposed [128, 2, N] operand tables, gather anchor rows, compute
    fp64 positive-pair dots from the unquantized values.
  - Device (per core): fp8 DoubleRow matmuls (one instruction per K=256
    contraction, 0.5 cycles/row) of its 1024 simclr rows and 512 spatial rows
    against the full 8192-column tables. The exp(x/(64T)) + row-sum work is
    split across TWO engines: the ACT engine (exact exp, accum_out) handles
    33 of the 48 2048-column groups, the DVE the other 15 via a
    two-instruction custom-op chain: p3(x)^2 (cubic Taylor of e^(sigma*x),
    then square) followed by 7 chained squarings -> p3(x)^256 ~ e^(x/(64T)),
    rel err ~1e-5, with a fused row-sum accumulator.
  - Host: computes each row's self-similarity term from the same quantized
    bytes, pushes it through the matching engine's exp formula, subtracts
    (near-exact cancellation of the huge diagonal terms), assembles sum_exp,
    log, subtracts positives, mean-reduces -> [2] losses.

Self-contained: hardcodes shapes from the problem spec.
"""
import sys
from operator import add as _op_add

for _p in ("/opt/trn_rl_repo", "/root/.axon_site/_ro/trn_rl_repo"):
    if _p not in sys.path:
        sys.path.insert(0, _p)

import numpy as np
import ml_dtypes

import concourse.tile as tile
from concourse import bacc, mybir
from concourse.bass_utils import run_bass_kernel_spmd

TEMPERATURE = 0.07
B = 4096     # simclr batch
D = 256      # projection dim
N = 8192     # num cells (spatial table rows, also 2B simclr table rows)
P = 4096     # num spatial pairs
NCORES = 8
SR = B // NCORES          # 512 simclr pair-rows per core (=> 1024 sim rows)
PR = P // NCORES          # 512 spatial rows per core
RT_SIMCLR = (2 * SR) // 128   # 8 row-tiles
RT_SPATIAL = PR // 128        # 4 row-tiles
RT_TOTAL = RT_SIMCLR + RT_SPATIAL  # 12
NGROUP = 4                # psum groups of 2048 columns
F32 = mybir.dt.float32
FP8 = mybir.dt.float8e4
NP_FP8 = ml_dtypes.float8_e4m3

# exp scaling: psum x = 64*(a.b) (8x-scaled fp8 rows); logit L = x*INV64T.
INV64T = float(1.0 / (64.0 * np.float32(TEMPERATURE)))
SIGMA = INV64T / 256.0    # DVE op1 approximates e^(SIGMA*x), chain raises ^256
D1 = float(np.float32(SIGMA))
D2 = float(np.float32(SIGMA * SIGMA / 2.0))
D3 = float(np.float32(SIGMA * SIGMA * SIGMA / 6.0))

# --- engine assignment: unit u = emission order over 48 (rt, g) units ------ #
# Emission: simclr group-major (g outer, rt 0..7 inner), then spatial
# (rt 8..11 outer, g inner). 15 DVE units spread evenly, none in the first
# two or last two slots (short ACT tail, fast ramp).
UNITS_ORDERED = [(rt, g) for g in range(NGROUP) for rt in range(RT_SIMCLR)]
UNITS_ORDERED += [(rt, g) for rt in range(RT_SIMCLR, RT_TOTAL) for g in range(NGROUP)]
_DVE_IDX = {2, 5, 8, 11, 14, 18, 21, 24, 27, 30, 33, 36, 39, 42, 45}
UNIT_ENGINE = {}          # (rt, g) -> "act" | "dve"
for _i, _u in enumerate(UNITS_ORDERED):
    UNIT_ENGINE[_u] = "dve" if _i in _DVE_IDX else "act"

_CACHE = {}


# --- custom DVE exp ops ----------------------------------------------------- #
def _register_dve_exp_ops():
    """Register two custom DVE ops (idempotent):
    EXP_P3_ANT:  out = p3(Src0)^2 with p3 = 1 + t + t^2/2 + t^3/6, t = SIGMA*x
                 folded as a Horner cubic in Src0 (C0=d3, C1=d2, C2=d1, +One).
    EXP_SQ8_ANT: out = Src0^128 via 7 chained squarings; accum_out = row sum.
    Chained: (p3(x)^2)^128 = p3(x)^256 ~ e^(256*SIGMA*x) = e^(x/(64T)).
    """
    import concourse.dve_ops as dve_ops
    from concourse.dve_ops import DveOp, OPS, CUSTOM_DVE_SPECS, _SUB_OPCODE_FOR_NAME
    from concourse.dve_spec import Spec, Src0, C0, C1, C2, One, lower
    from concourse.dve_uop import DveOpSpec

    if "EXP_P3_ANT" in CUSTOM_DVE_SPECS:
        return

    m1 = Src0 * C0
    a1 = m1 + C1
    m2 = a1 * Src0
    a2 = m2 + C2
    m3 = a2 * Src0
    p = m3 + One
    spec_p3 = Spec(body=p * p, reference=lambda *a: _host_exp_p3(a[0], a[2], a[3], a[4]))

    y = Src0
    for _ in range(7):
        y = y * y
    spec_sq8 = Spec(
        body=y, accum=_op_add,
        reference=lambda *a: _host_exp_sq8_accum(a[0]),
    )

    ops = []
    for name, spec in (("EXP_P3_ANT", spec_p3), ("EXP_SQ8_ANT", spec_sq8)):
        shas = {}
        for ver in ("v3", "v4"):
            uops = lower(spec, ver=ver)
            shas[ver] = DveOpSpec(name=name, opcode=0, uops=uops, rd1_en=False).sha(ver)
        ops.append(DveOp(name, spec, subdim=False, uops_sha=shas))

    base = max(_SUB_OPCODE_FOR_NAME.values()) + 1
    for i, op in enumerate(ops):
        OPS.append(op)
        CUSTOM_DVE_SPECS[op.name] = op.spec
        _SUB_OPCODE_FOR_NAME[op.name] = base + i
    dve_ops.EXP_P3_ANT = ops[0]
    dve_ops.EXP_SQ8_ANT = ops[1]


def _host_exp_p3(x, c0=D3, c1=D2, c2=D1):
    """fp32 reference for EXP_P3_ANT; also used host-side for corrections."""
    x = np.asarray(x).astype(np.float32)
    m1 = (x * np.float32(c0)).astype(np.float32)
    a1 = (m1 + np.float32(c1)).astype(np.float32)
    m2 = (a1 * x).astype(np.float32)
    a2 = (m2 + np.float32(c2)).astype(np.float32)
    m3 = (a2 * x).astype(np.float32)
    p = (m3 + np.float32(1.0)).astype(np.float32)
    return (p * p).astype(np.float32)


def _host_exp_sq7(x):
    y = np.asarray(x).astype(np.float32)
    for _ in range(7):
        y = (y * y).astype(np.float32)
    return y


def _host_exp_sq8_accum(in0):
    y = _host_exp_sq7(in0)
    acc = y.reshape(y.shape[0], -1).sum(axis=-1, dtype=np.float64)
    return y, acc.astype(np.float32).reshape(y.shape[0], 1)


def _get_exp_ops():
    _register_dve_exp_ops()
    import concourse.dve_ops as dve_ops

    return dve_ops.EXP_P3_ANT, dve_ops.EXP_SQ8_ANT


def _build_nc():
    EXP_P3, EXP_SQ8 = _get_exp_ops()
    nc = bacc.Bacc("TRN2", target_bir_lowering=False)

    zT = nc.dram_tensor("zT", [128, 2, N], FP8, kind="ExternalInput")
    eT = nc.dram_tensor("eT", [128, 2, N], FP8, kind="ExternalInput")
    zTl = nc.dram_tensor("zTl", [128, 2, 2 * SR], FP8, kind="ExternalInput")
    aTl = nc.dram_tensor("aTl", [128, 2, PR], FP8, kind="ExternalInput")

    sraw_o = nc.dram_tensor("sraw", [128, RT_TOTAL], F32, kind="ExternalOutput")

    with tile.TileContext(nc) as tc:
        with (
            tc.tile_pool(name="tabs", bufs=1) as tabs,
            tc.tile_pool(name="psum", bufs=2, space="PSUM") as psum,
            tc.tile_pool(name="qscr", bufs=2) as qscr,
            tc.tile_pool(name="small", bufs=1) as small,
        ):
            zTl_t = tabs.tile([128, 2, 2 * SR], FP8)
            aTl_t = tabs.tile([128, 2, PR], FP8)
            # Group 0 of zT in four 512-column sub-tiles so the very first
            # matmuls/exps only wait for a small DMA.
            zT_c = [tabs.tile([128, 2, 512], FP8, name=f"zTc{j}") for j in range(4)]
            zT_g = [None] + [tabs.tile([128, 2, 2048], FP8, name=f"zTg{g}")
                             for g in range(1, NGROUP)]
            eT_g = [tabs.tile([128, 2, 2048], FP8, name=f"eTg{g}")
                    for g in range(NGROUP)]
            # DMA issue order = consumption order (single serial HWDGE queue):
            # local simclr rows, first column chunks, remaining z groups, then
            # the spatial tables (needed only from unit 32 on).
            nc.sync.dma_start(zTl_t[:], zTl[:])
            for j in range(4):
                nc.sync.dma_start(zT_c[j][:], zT[:, :, j * 512:(j + 1) * 512])
            for g in range(1, NGROUP):
                nc.sync.dma_start(zT_g[g][:], zT[:, :, g * 2048:(g + 1) * 2048])
            nc.sync.dma_start(aTl_t[:], aTl[:])
            for g in range(NGROUP):
                nc.sync.dma_start(eT_g[g][:], eT[:, :, g * 2048:(g + 1) * 2048])

            sraw_t = small.tile([128, RT_TOTAL], F32)

            def lhsT_pair(rt):
                lh, li = (zTl_t, rt) if rt < RT_SIMCLR else (aTl_t, rt - RT_SIMCLR)
                return lh[:, :, li * 128:(li + 1) * 128]

            # Persistent per-(row-tile, group) partial sums; zeroed once so
            # the final reduce can span unused slots of the fine-grained rt0.
            part_all = small.tile([128, RT_TOTAL, NGROUP + 3], F32)
            nc.vector.memset(part_all[:], 0.0)

            def emit_unit(rt, g):
                """4 DoubleRow matmuls + exp(accum) for one (rt, 2048-col group)."""
                lz = lhsT_pair(rt)
                simclr = rt < RT_SIMCLR
                fine = rt == 0 and g == 0
                engine = UNIT_ENGINE[(rt, g)]
                pg = psum.tile([128, 2048], F32, tag="big")
                for cc in range(4):
                    if simclr and g == 0:
                        rhs = zT_c[cc][:]
                    else:
                        tab = zT_g[g] if simclr else eT_g[g]
                        rhs = tab[:, :, cc * 512:(cc + 1) * 512]
                    nc.tensor.matmul(
                        pg[:, cc * 512:(cc + 1) * 512], lz, rhs,
                        start=True, stop=True,
                        perf_mode=mybir.MatmulPerfMode.DoubleRow,
                    )
                if engine == "act":
                    # exp output is dead (only accum_out matters): write it
                    # in-place over the PSUM bank.
                    if fine:
                        for cc in range(4):
                            nc.scalar.activation(
                                pg[:, cc * 512:(cc + 1) * 512],
                                pg[:, cc * 512:(cc + 1) * 512],
                                mybir.ActivationFunctionType.Exp,
                                scale=INV64T,
                                accum_out=part_all[:, rt, cc:cc + 1],
                            )
                    else:
                        ps = g + 3 if rt == 0 else g
                        nc.scalar.activation(
                            pg[:], pg[:], mybir.ActivationFunctionType.Exp,
                            scale=INV64T, accum_out=part_all[:, rt, ps:ps + 1],
                        )
                else:
                    ps = g + 3 if rt == 0 else g
                    q = qscr.tile([128, 2048], F32, tag="q")
                    nc.vector._custom_dve(EXP_P3, out=q[:], in0=pg[:],
                                          s0=D3, s1=D2, imm2=D1)
                    nc.vector._custom_dve(EXP_SQ8, out=q[:], in0=q[:],
                                          accum_out=part_all[:, rt, ps:ps + 1])

            for rt, g in UNITS_ORDERED:
                emit_unit(rt, g)

            nc.vector.tensor_reduce(
                sraw_t[:], part_all[:],
                axis=mybir.AxisListType.X, op=mybir.AluOpType.add,
            )
            nc.sync.dma_start(sraw_o[:], sraw_t[:])

    nc.finalize()
    return nc


def _l2norm(x):
    n = np.maximum(np.linalg.norm(x.astype(np.float32), axis=1, keepdims=True), 1e-12)
    return (x.astype(np.float32) / n).astype(np.float32)


def _pack_T(x8):
    """[R, D=256] fp8 -> transposed operand table [128, 2, R] (k-tile dim 1)."""
    xT = np.ascontiguousarray(x8.T)                     # [256, R]
    return np.ascontiguousarray(
        xT.reshape(2, 128, xT.shape[1]).transpose(1, 0, 2)
    )


def _self_dots(q8):
    """Emulate the device's self-similarity matmul element for each quantized
    row: fp32 accumulation per 128-wide k-tile, tile0 + tile1."""
    q = q8.astype(np.float32)
    d0 = np.einsum("ij,ij->i", q[:, :128], q[:, :128], dtype=np.float32)
    d1 = np.einsum("ij,ij->i", q[:, 128:], q[:, 128:], dtype=np.float32)
    return (d0.astype(np.float32) + d1.astype(np.float32)).astype(np.float32)


def _corr_both(d):
    """(corr_act, corr_dve): the two engines' exp of the self term."""
    corr_a = np.exp((d * np.float32(INV64T)).astype(np.float32))
    corr_d = _host_exp_sq7(_host_exp_p3(d))
    return corr_a.astype(np.float64), corr_d.astype(np.float64)


def prepare(z1, z2, embeddings, anchor_idx, neighbor_idx):
    """Host-side prep: returns (in_maps, host_ctx)."""
    z1n = _l2norm(np.asarray(z1))
    z2n = _l2norm(np.asarray(z2))
    en = _l2norm(np.asarray(embeddings))
    ai = np.asarray(anchor_idx).astype(np.int64)
    ni = np.asarray(neighbor_idx).astype(np.int64)

    zcat8 = (np.concatenate([z1n, z2n], axis=0) * np.float32(8.0)).astype(NP_FP8)
    en8 = (en * np.float32(8.0)).astype(NP_FP8)
    zT_p = _pack_T(zcat8)                               # [128, 2, 8192] fp8
    eT_p = _pack_T(en8)                                 # [128, 2, 8192] fp8
    a8 = en8[ai]                                        # [P, D] fp8 (table rows)
    aT_p = _pack_T(a8)                                  # [128, 2, 4096] fp8

    # fp64 positive-pair logits from the unquantized values
    psim = (np.sum(z1n.astype(np.float64) * z2n.astype(np.float64), axis=1)
            / np.float64(np.float32(TEMPERATURE)))      # [B]
    pos = (np.sum(en[ai].astype(np.float64) * en[ni].astype(np.float64), axis=1)
           / np.float64(np.float32(TEMPERATURE)))       # [P]
    eq = (ai == ni).astype(np.float64)                  # [P]

    # Self-term corrections (host-side, both engine variants)
    ca_z, cd_z = _corr_both(_self_dots(zcat8))          # [2B]
    ca_a, cd_a = _corr_both(_self_dots(a8))             # [P]

    in_maps = []
    for c in range(NCORES):
        zTl_p = np.ascontiguousarray(np.concatenate(
            [zT_p[:, :, c * SR:(c + 1) * SR],
             zT_p[:, :, B + c * SR:B + (c + 1) * SR]], axis=2))  # [128,2,1024]
        aTl_p = np.ascontiguousarray(aT_p[:, :, c * PR:(c + 1) * PR])  # [128,2,512]
        in_maps.append({"zT": zT_p, "eT": eT_p, "zTl": zTl_p, "aTl": aTl_p})
    return in_maps, (psim, pos, eq, ai, ca_z, cd_z, ca_a, cd_a)


def finish(results, host_ctx):
    """Host-side epilogue: assemble the two losses from per-core row sums."""
    psim, pos, eq, ai, ca_z, cd_z, ca_a, cd_a = host_ctx
    terms1 = np.empty(2 * B, dtype=np.float64)
    terms2 = np.empty(P, dtype=np.float64)
    l_sim = np.arange(2 * SR)
    rt_sim = l_sim // 128
    r_sp = np.arange(PR)
    rt_sp = RT_SIMCLR + r_sp // 128
    for c in range(NCORES):
        S = results[c]["sraw"].astype(np.float64).T.reshape(-1)   # idx rt*128+p

        # simclr local rows l in [0, 1024): global z-table col
        gcol = np.where(l_sim < SR, c * SR + l_sim, B + c * SR + (l_sim - SR))
        g_of_col = gcol // 2048
        use_dve = np.array(
            [UNIT_ENGINE[(int(rt_sim[i]), int(g_of_col[i]))] == "dve"
             for i in range(2 * SR)])
        corr_sim = np.where(use_dve, cd_z[gcol], ca_z[gcol])
        sum_exp = S[:2 * SR] - corr_sim
        p_loc = psim[c * SR:(c + 1) * SR]
        terms1[c * SR:(c + 1) * SR] = np.log(sum_exp[:SR]) - p_loc
        terms1[B + c * SR:B + (c + 1) * SR] = np.log(sum_exp[SR:2 * SR]) - p_loc

        # spatial local rows r in [0, 512): anchor col is data-dependent
        gsl = slice(c * PR, (c + 1) * PR)
        ai_loc = ai[gsl]
        g_anchor = ai_loc // 2048
        use_dve_sp = np.array(
            [UNIT_ENGINE[(int(rt_sp[i]), int(g_anchor[i]))] == "dve"
             for i in range(PR)])
        s_sp = S[2 * SR:2 * SR + PR]
        c_sp = np.where(use_dve_sp, cd_a[gsl], ca_a[gsl])
        total = s_sp - c_sp + eq[gsl] * np.exp(pos[gsl])
        terms2[gsl] = np.log(total) - pos[gsl]

    l1 = terms1.mean()
    l2 = terms2.mean()
    return np.array([l1, l2], dtype=np.float32)


def get_nc():
    if "nc" not in _CACHE:
        _CACHE["nc"] = _build_nc()
    return _CACHE["nc"]


def kernel(z1, z2, embeddings, anchor_idx, neighbor_idx):
    in_maps, host_ctx = prepare(z1, z2, embeddings, anchor_idx, neighbor_idx)
    nc = get_nc()
    res = run_bass_kernel_spmd(nc, in_maps, list(range(NCORES)))
    return finish(res.results, host_ctx)
